# revision 30
# baseline (speedup 1.0000x reference)
"""AdaptiveTopKChannelStack (SG-MoE noisy-gate monotonic top-k) on 8 TRN2 NeuronCores.

Data-parallel over batch: each core handles 4096 of the 32768 rows.
Per core:
  - gate:  H = x @ Wg + noise_eps * softplus(x @ Wn). The gate matmul runs
           as a 3-term bf16 hi/lo split (x = xh + xl, W = Wh + Wl host-side;
           xh@Wh + xl@Wh + xh@Wl accumulate in fp32 PSUM; the dropped
           xl@Wl term is O(2^-18)), giving |H err| ~4e-5, below the
           minimum top-2 gap of H, so the argmax matches the fp32
           reference exactly while keeping the whole PE stream bf16
           (mixing fp32 and bf16 matmuls trips a hardware FWL/FP32
           hazard that faults the exec unit). The 3 tiny gate matmuls per
           chunk ride inside the main weight stream so their (duplicated;
           walrus runs with ldw-opt off) LDWEIGHTS hide under the 512-wide
           main streams via the PE's reorder window.
           softplus = relu(z) + ln(1 + exp(-|z|)): relu on DVE
           (tensor_scalar max), abs/exp/ln on the scalar engine, the two
           SBUF-only adds/mults on GpSimd.
  - route: k = argmax(H) via prefix-max scan; threshold t = 128*(k+1)
  - main:  y = x @ Wc_flat  (bf16 = xh @ Wc, fp32 accumulate in PSUM)
  - epilogue, spread across engines for DVE fast modes (the fused
    scalar_tensor_tensor supports no 2x/4x mode and is 2.6x slower):
           mask16 = (iota16 < t)     DVE tensor_scalar, 4x_2p, fp16
           y16    = downcast(psum)   scalar engine activation-copy, fp16
           out    = mask16 * y16     DVE tensor_tensor, 2x_1p
    fp16 keeps iota exact to 2048 (bf16 would round above 1024) and
    halves the output DMA vs fp32 (rel-err contribution ~2e-4).
x is transposed host-side so the contraction dim lands on SBUF partitions
with fully contiguous DMA; weights are replicated to all cores. Input
x/wc DMAs are batched (3D APs) and ordered so tile 0 can start ~5us in;
the per-tile output is one 2048-wide fp16 DMA on the sync queue.
"""

import numpy as np
import ml_dtypes

import concourse.bass as bass
import concourse.mybir as mybir
import concourse.tile as tile
from concourse.bass_utils import run_bass_kernel_spmd

F32 = mybir.dt.float32
BF16 = mybir.dt.bfloat16
F16 = mybir.dt.float16
I32 = mybir.dt.int32
AF = mybir.ActivationFunctionType
OP = mybir.AluOpType

N_CORES = 8
B, D, E, CH = 32768, 512, 16, 128
NF = E * CH              # 2048 out features
BLOC = B // N_CORES      # 4096 rows per core
KC = D // 128            # 4 contraction chunks
NT = BLOC // 128         # 32 row tiles per core
XBLK = 1024              # x DMA column granularity


def _split_multi_waits(nc, max_waits=1):
    """walrus rejects instructions with more than a couple of semaphore
    waits; hoist extra waits into single-wait NOPs ahead of the instruction
    (same engine executes in order, so semantics are unchanged)."""
    for f in nc.m.functions:
        for bb in f.blocks:
            new_insts = []
            for inst in bb.instructions:
                si = inst.sync_info
                if si is not None and si.on_wait and len(si.on_wait) > max_waits:
                    waits = list(si.on_wait)
                    for j, w in enumerate(waits[max_waits:]):
                        new_insts.append(mybir.InstNoOp(
                            name=f"{inst.name}-waitsplit-{j}",
                            sync_info=mybir.SyncInfo(on_wait=[w], on_update=[]),
                            bass_nofuse=True,
                            engine=inst.engine,
                        ))
                    si.on_wait = waits[:max_waits]
                new_insts.append(inst)
            bb.instructions[:] = new_insts


def _build(has_gate_bias, has_comp_bias, split_waits=True):
    nc = bass.Bass("TRN2", target_bir_lowering=False, debug=False)

    xh_ext = nc.declare_dram_parameter("xh", [D, BLOC], BF16, isOutput=False)
    xl_ext = nc.declare_dram_parameter("xl", [D, BLOC], BF16, isOutput=False)
    wc_ext = nc.declare_dram_parameter("wc", [D, NF], BF16, isOutput=False)
    # [Wh | Wl] hi/lo split of [Wg|Wn], concatenated along the free dim
    whl_ext = nc.declare_dram_parameter("whl", [D, 4 * E], BF16, isOutput=False)
    eps_ext = nc.declare_dram_parameter("eps", [1, E], F32, isOutput=False)
    if has_gate_bias:
        gb_ext = nc.declare_dram_parameter("gb", [1, 2 * E], F32, isOutput=False)
    if has_comp_bias:
        bc_ext = nc.declare_dram_parameter("bc", [1, NF], F32, isOutput=False)
    out_ext = nc.declare_dram_parameter("out", [BLOC, NF], F16, isOutput=True)

    with tile.TileContext(nc) as tc:
        with (
            tc.tile_pool(name="big", bufs=1) as big,
            tc.tile_pool(name="outp", bufs=6) as outp,
            # 12 untagged slots: the ~10 tiny gate tiles per iteration
            # otherwise share 3 slots and serialize across iterations on
            # WAR hazards (slots are 64B/partition, so this is free)
            tc.tile_pool(name="small", bufs=12) as small,
            tc.tile_pool(name="ps", bufs=3, space="PSUM") as ps,
            tc.tile_pool(name="psg", bufs=2, space="PSUM") as psg,
        ):
            # ---- resident tensors ----
            xh_t = big.tile([128, KC, BLOC], BF16)
            xl_t = big.tile([128, KC, BLOC], BF16)
            wc_t = big.tile([128, KC, NF], BF16)
            whl_t = big.tile([128, KC, 4 * E], BF16)
            eps_t = big.tile([128, E], F32)
            iota_i = big.tile([128, NF], I32)
            # fp16 iota (integers <= 2048 are exact in fp16; bf16 would
            # round 1024..2047 to multiples of 8, corrupting the compare)
            iota_h = big.tile([128, NF], F16)

            nc.sync.dma_start(out=whl_t, in_=whl_ext.ap().rearrange(
                "(c p) e -> p c e", p=128))
            eps_bc = bass.AP(tensor=eps_ext, offset=0, ap=[[0, 128], [1, E]])
            nc.gpsimd.dma_start(out=eps_t, in_=eps_bc)
            if has_gate_bias:
                gb_t = big.tile([128, 2 * E], F32)
                nc.gpsimd.dma_start(out=gb_t, in_=bass.AP(
                    tensor=gb_ext, offset=0, ap=[[0, 128], [1, 2 * E]]))
            if has_comp_bias:
                bc_t = big.tile([128, NF], F32)
                nc.gpsimd.dma_start(out=bc_t, in_=bass.AP(
                    tensor=bc_ext, offset=0, ap=[[0, 128], [1, NF]]))
            nc.gpsimd.iota(iota_i, pattern=[[1, NF]], base=0, channel_multiplier=0)
            nc.gpsimd.tensor_copy(iota_h, iota_i)
            # x block 0 lands first so tile 0 can start, then wc arrives
            # per chunk in consumption order (the bulk ring is in-order,
            # so tile 0 waits for ~1.3MB, not the whole 2MB of wc), then
            # the remaining x blocks. Batched DMAs — the sync queue is
            # issue-rate-bound at ~750ns per dma_start.
            xh_r = xh_ext.ap().rearrange("(c p) e -> p c e", p=128)
            xl_r = xl_ext.ap().rearrange("(c p) e -> p c e", p=128)
            wc_r = wc_ext.ap().rearrange("(c p) e -> p c e", p=128)
            xblocks = [256, 256, 512] + [XBLK] * ((BLOC - 1024) // XBLK)
            start = 0
            for j, blk in enumerate(xblocks):
                cols = slice(start, start + blk)
                start += blk
                nc.sync.dma_start(out=xh_t[:, :, cols], in_=xh_r[:, :, cols])
                nc.sync.dma_start(out=xl_t[:, :, cols], in_=xl_r[:, :, cols])
                if j == 0:
                    for c in range(KC):
                        nc.sync.dma_start(out=wc_t[:, c, :],
                                          in_=wc_r[:, c, :])

            # ---- per-row-tile pipeline ----
            for i in range(NT):
                rows = slice(i * 128, (i + 1) * 128)

                # gate accumulates 3 bf16 hi/lo terms (xh@Wh + xh@Wl + xl@Wh;
                # the dropped xl@Wl term is O(2^-18)) into ps_g. The two xh
                # terms ride inside the main weight stream so their LDWEIGHTS
                # is the one already loaded for the 512-wide main matmuls;
                # only the 4 xl chunks pay a (reorder-window-hidden) reload.
                ps_g = psg.tile([128, 2 * E], F32)
                ps_h = [ps.tile([128, 1024], F32, tag="ps_half",
                                name=f"ps_half_{i}_{h}") for h in range(2)]
                # single interleaved pass: per chunk, 4x512-wide main
                # streams keep the PE ahead of the (duplicated, walrus
                # runs with ldw-opt off) LDWEIGHTS; the 3 tiny gate
                # matmuls ride along. (A split h0/h1 two-pass variant
                # measured 29% SLOWER: its half-length streams can't hide
                # 5 LDWs per chunk.)
                for c in range(KC):
                    for n in range(4):
                        nc.tensor.matmul(
                            ps_h[n // 2][:, (n % 2) * 512:(n % 2) * 512 + 512],
                            xh_t[:, c, rows],
                            wc_t[:, c, n * 512:(n + 1) * 512],
                            start=(c == 0), stop=(c == KC - 1))
                    nc.tensor.matmul(
                        ps_g[:, :], xh_t[:, c, rows],
                        whl_t[:, c, 0:2 * E],
                        start=(c == 0), stop=False)
                    nc.tensor.matmul(
                        ps_g[:, :], xh_t[:, c, rows],
                        whl_t[:, c, 2 * E:4 * E],
                        start=False, stop=False)
                    nc.tensor.matmul(
                        ps_g[:, :], xl_t[:, c, rows],
                        whl_t[:, c, 0:2 * E],
                        start=False, stop=(c == KC - 1))

                # h0 copy first on the scalar queue (frees psum buf early),
                # then the gate epilogue overlapping pass 2.
                y16_t = small.tile([128, NF], F16, tag="y16", bufs=3,
                                   name=f"y16_{i}")

                def copy_half(h):
                    src = ps_h[h]
                    if has_comp_bias:
                        src = small.tile([128, 1024], F32, tag="biased", bufs=3,
                                         name=f"biased_{i}_{h}")
                        nc.vector.tensor_tensor(
                            out=src, in0=ps_h[h],
                            in1=bc_t[:, h * 1024:(h + 1) * 1024], op=OP.add)
                    nc.scalar.activation(y16_t[:, h * 1024:(h + 1) * 1024],
                                         src, AF.Copy)

                copy_half(0)
                copy_half(1)

                # gate epilogue -> threshold t = 128*(argmax+1)
                g_ps = ps_g[:, 0:E]
                n_ps = ps_g[:, E:2 * E]
                if has_gate_bias:
                    gn_t = small.tile([128, 2 * E], F32)
                    nc.vector.tensor_tensor(out=gn_t, in0=ps_g, in1=gb_t,
                                            op=OP.add)
                    g_ps, n_ps = gn_t[:, 0:E], gn_t[:, E:2 * E]
                # relu(z) as a single DVE tensor_scalar (max with 0) so the
                # scalar engine only carries Abs/Exp/Ln plus the big copies
                re_t = small.tile([128, E], F32)
                nc.vector.tensor_scalar(out=re_t, in0=n_ps, scalar1=0.0,
                                        scalar2=None, op0=OP.max)
                ab_t = small.tile([128, E], F32)
                nc.scalar.activation(ab_t, n_ps, AF.Abs)
                ex_t = small.tile([128, E], F32)
                nc.scalar.activation(ex_t, ab_t, AF.Exp, scale=-1.0)
                ln_t = small.tile([128, E], F32)
                nc.scalar.activation(ln_t, ex_t, AF.Ln, bias=1.0)
                # the two SBUF-only small ops go to GpSimd to unload DVE
                sp_t = small.tile([128, E], F32)
                nc.gpsimd.tensor_tensor(out=sp_t, in0=ln_t, in1=re_t, op=OP.add)
                he_t = small.tile([128, E], F32)
                nc.gpsimd.tensor_tensor(out=he_t, in0=sp_t, in1=eps_t, op=OP.mult)
                h_t = small.tile([128, E], F32)
                nc.vector.tensor_tensor(out=h_t, in0=he_t, in1=g_ps, op=OP.add)
                pm_t = small.tile([128, E], F32)
                nc.vector.tensor_tensor_scan(pm_t, h_t, h_t, initial=-1e30,
                                             op0=OP.max, op1=OP.bypass)
                bits_t = small.tile([128, E], F32)
                ks_t = small.tile([128, 1], F32)
                nc.vector.tensor_scalar(out=bits_t, in0=pm_t,
                                        scalar1=pm_t[:, E - 1:E], scalar2=0.0,
                                        op0=OP.is_lt, op1=OP.add, accum_out=ks_t)
                t_t = small.tile([128, 1], F32)
                nc.vector.tensor_scalar(out=t_t, in0=ks_t, scalar1=128.0,
                                        scalar2=128.0, op0=OP.mult, op1=OP.add)


                # masked epilogue, restructured for DVE fast modes (the
                # fused scalar_tensor_tensor supports NO 2x/4x mode and
                # costs 1.5us per half from PSUM):
                #   mask16 = (iota16 < t)          DVE tensor_scalar, 4x_2p
                #   y16    = downcast(psum)        scalar engine, x2
                #   out    = mask16 * y16          DVE tensor_tensor, 2x_1p
                mask_t = small.tile([128, NF], F16, tag="mask", bufs=3,
                                    name=f"mask_{i}")
                nc.vector.tensor_scalar(out=mask_t, in0=iota_h,
                                        scalar1=t_t[:, 0:1], scalar2=None,
                                        op0=OP.is_lt)
                o_t = outp.tile([128, NF], F16)
                if i < NT - 1:
                    nc.vector.tensor_tensor(out=o_t, in0=mask_t, in1=y16_t,
                                            op=OP.mult)
                    nc.sync.dma_start(out=out_ext[rows, :], in_=o_t)
                else:
                    # last tile: split halves so h0's mult+DMA overlap the
                    # h1 psum copy, shortening the kernel tail
                    for h in range(2):
                        hc = slice(h * 1024, (h + 1) * 1024)
                        nc.vector.tensor_tensor(out=o_t[:, hc],
                                                in0=mask_t[:, hc],
                                                in1=y16_t[:, hc], op=OP.mult)
                        nc.sync.dma_start(out=out_ext[rows, hc],
                                          in_=o_t[:, hc])

    if split_waits:
        _split_multi_waits(nc)
    return nc


_NC_CACHE = {}


def kernel(x, Wc, bc, Wg_w, Wg_b, Wn_w, Wn_b, noise_eps):
    x = np.ascontiguousarray(np.asarray(x, dtype=np.float32))
    Wc = np.asarray(Wc, dtype=np.float32)
    bc = np.asarray(bc, dtype=np.float32)
    Wg_w = np.asarray(Wg_w, dtype=np.float32)
    Wg_b = np.asarray(Wg_b, dtype=np.float32)
    Wn_w = np.asarray(Wn_w, dtype=np.float32)
    Wn_b = np.asarray(Wn_b, dtype=np.float32)
    noise_eps = np.asarray(noise_eps, dtype=np.float32)

    has_gate_bias = bool(np.any(Wg_b) or np.any(Wn_b))
    has_comp_bias = bool(np.any(bc))

    key = (has_gate_bias, has_comp_bias)
    if key not in _NC_CACHE:
        _NC_CACHE[key] = _build(has_gate_bias, has_comp_bias)
    nc = _NC_CACHE[key]

    bf = ml_dtypes.bfloat16
    xT = x.T                                   # [D, B]
    xh = xT.astype(bf)
    xl = (xT - xh.astype(np.float32)).astype(bf)
    wgn = np.concatenate([Wg_w, Wn_w], axis=1)  # [D, 2E] fp32
    wgh = wgn.astype(bf)
    wgl = (wgn - wgh.astype(np.float32)).astype(bf)
    whl = np.ascontiguousarray(np.concatenate([wgh, wgl], axis=1))  # [D, 4E]
    wc_flat = np.ascontiguousarray(
        Wc.transpose(1, 0, 2).reshape(D, NF).astype(bf))
    eps2 = np.ascontiguousarray(noise_eps.reshape(1, E))

    in_maps = []
    for i in range(N_CORES):
        cols = slice(i * BLOC, (i + 1) * BLOC)
        m = {
            "xh": np.ascontiguousarray(xh[:, cols]),
            "xl": np.ascontiguousarray(xl[:, cols]),
            "wc": wc_flat,
            "whl": whl,
            "eps": eps2,
        }
        if has_gate_bias:
            m["gb"] = np.ascontiguousarray(
                np.concatenate([Wg_b, Wn_b]).reshape(1, 2 * E).astype(np.float32))
        if has_comp_bias:
            m["bc"] = np.ascontiguousarray(bc.reshape(1, NF).astype(np.float32))
        in_maps.append(m)

    res = run_bass_kernel_spmd(nc, in_maps, core_ids=list(range(N_CORES)))
    out = np.concatenate(
        [np.asarray(res.results[i]["out"]).astype(np.float32)
         for i in range(N_CORES)], axis=0)
    return out



# revision 32
# speedup vs baseline: 1.0037x; 1.0037x over previous
"""AdaptiveTopKChannelStack (SG-MoE noisy-gate monotonic top-k) on 8 TRN2 NeuronCores.

Data-parallel over batch: each core handles 4096 of the 32768 rows.
Per core:
  - gate:  H = x @ Wg + noise_eps * softplus(x @ Wn). The gate matmul runs
           as a 3-term bf16 hi/lo split (x = xh + xl, W = Wh + Wl host-side;
           xh@Wh + xl@Wh + xh@Wl accumulate in fp32 PSUM; the dropped
           xl@Wl term is O(2^-18)), giving |H err| ~4e-5, below the
           minimum top-2 gap of H, so the argmax matches the fp32
           reference exactly while keeping the whole PE stream bf16
           (mixing fp32 and bf16 matmuls trips a hardware FWL/FP32
           hazard that faults the exec unit). The 3 tiny gate matmuls per
           chunk ride inside the main weight stream so their (duplicated;
           walrus runs with ldw-opt off) LDWEIGHTS hide under the 512-wide
           main streams via the PE's reorder window.
           softplus = relu(z) + ln(1 + exp(-|z|)): relu on DVE
           (tensor_scalar max), abs/exp/ln on the scalar engine, the two
           SBUF-only adds/mults on GpSimd.
  - route: k = argmax(H) via prefix-max scan; threshold t = 128*(k+1)
  - main:  y = x @ Wc_flat  (bf16 = xh @ Wc, fp32 accumulate in PSUM)
  - epilogue, spread across engines for DVE fast modes (the fused
    scalar_tensor_tensor supports no 2x/4x mode and is 2.6x slower):
           mask16 = (iota16 < t)     DVE tensor_scalar, 4x_2p, fp16
           y16    = downcast(psum)   scalar engine activation-copy, fp16
           out    = mask16 * y16     DVE tensor_tensor, 2x_1p
    fp16 keeps iota exact to 2048 (bf16 would round above 1024) and
    halves the output DMA vs fp32 (rel-err contribution ~2e-4).
x is transposed host-side so the contraction dim lands on SBUF partitions
with fully contiguous DMA; weights are replicated to all cores. Input
x/wc DMAs are batched (3D APs) and ordered so tile 0 can start ~5us in;
the per-tile output is one 2048-wide fp16 DMA on the sync queue.
"""

import numpy as np
import ml_dtypes

import concourse.bass as bass
import concourse.mybir as mybir
import concourse.tile as tile
from concourse.bass_utils import run_bass_kernel_spmd

F32 = mybir.dt.float32
BF16 = mybir.dt.bfloat16
F16 = mybir.dt.float16
I32 = mybir.dt.int32
AF = mybir.ActivationFunctionType
OP = mybir.AluOpType

N_CORES = 8
B, D, E, CH = 32768, 512, 16, 128
NF = E * CH              # 2048 out features
BLOC = B // N_CORES      # 4096 rows per core
KC = D // 128            # 4 contraction chunks
NT = BLOC // 128         # 32 row tiles per core
XBLK = 1024              # x DMA column granularity


def _split_multi_waits(nc, max_waits=1):
    """walrus rejects instructions with more than a couple of semaphore
    waits; hoist extra waits into single-wait NOPs ahead of the instruction
    (same engine executes in order, so semantics are unchanged)."""
    for f in nc.m.functions:
        for bb in f.blocks:
            new_insts = []
            for inst in bb.instructions:
                si = inst.sync_info
                if si is not None and si.on_wait and len(si.on_wait) > max_waits:
                    waits = list(si.on_wait)
                    for j, w in enumerate(waits[max_waits:]):
                        new_insts.append(mybir.InstNoOp(
                            name=f"{inst.name}-waitsplit-{j}",
                            sync_info=mybir.SyncInfo(on_wait=[w], on_update=[]),
                            bass_nofuse=True,
                            engine=inst.engine,
                        ))
                    si.on_wait = waits[:max_waits]
                new_insts.append(inst)
            bb.instructions[:] = new_insts


def _build(has_gate_bias, has_comp_bias, split_waits=True):
    nc = bass.Bass("TRN2", target_bir_lowering=False, debug=False)

    xh_ext = nc.declare_dram_parameter("xh", [D, BLOC], BF16, isOutput=False)
    xl_ext = nc.declare_dram_parameter("xl", [D, BLOC], BF16, isOutput=False)
    wc_ext = nc.declare_dram_parameter("wc", [D, NF], BF16, isOutput=False)
    # [Wh | Wl] hi/lo split of [Wg|Wn], concatenated along the free dim
    whl_ext = nc.declare_dram_parameter("whl", [D, 4 * E], BF16, isOutput=False)
    eps_ext = nc.declare_dram_parameter("eps", [1, E], F32, isOutput=False)
    if has_gate_bias:
        gb_ext = nc.declare_dram_parameter("gb", [1, 2 * E], F32, isOutput=False)
    if has_comp_bias:
        bc_ext = nc.declare_dram_parameter("bc", [1, NF], F32, isOutput=False)
    out_ext = nc.declare_dram_parameter("out", [BLOC, NF], F16, isOutput=True)

    with tile.TileContext(nc) as tc:
        with (
            tc.tile_pool(name="big", bufs=1) as big,
            tc.tile_pool(name="outp", bufs=5) as outp,
            # 12 untagged slots: the ~10 tiny gate tiles per iteration
            # otherwise share 3 slots and serialize across iterations on
            # WAR hazards (slots are 64B/partition, so this is free)
            tc.tile_pool(name="small", bufs=12) as small,
            tc.tile_pool(name="ps", bufs=3, space="PSUM") as ps,
            tc.tile_pool(name="psg", bufs=2, space="PSUM") as psg,
        ):
            # ---- resident tensors ----
            xh_t = big.tile([128, KC, BLOC], BF16)
            xl_t = big.tile([128, KC, BLOC], BF16)
            wc_t = big.tile([128, KC, NF], BF16)
            whl_t = big.tile([128, KC, 4 * E], BF16)
            eps_t = big.tile([128, E], F32)
            iota_i = big.tile([128, NF], I32)
            # fp16 iota (integers <= 2048 are exact in fp16; bf16 would
            # round 1024..2047 to multiples of 8, corrupting the compare)
            iota_h = big.tile([128, NF], F16)

            nc.sync.dma_start(out=whl_t, in_=whl_ext.ap().rearrange(
                "(c p) e -> p c e", p=128))
            eps_bc = bass.AP(tensor=eps_ext, offset=0, ap=[[0, 128], [1, E]])
            nc.gpsimd.dma_start(out=eps_t, in_=eps_bc)
            if has_gate_bias:
                gb_t = big.tile([128, 2 * E], F32)
                nc.gpsimd.dma_start(out=gb_t, in_=bass.AP(
                    tensor=gb_ext, offset=0, ap=[[0, 128], [1, 2 * E]]))
            if has_comp_bias:
                bc_t = big.tile([128, NF], F32)
                nc.gpsimd.dma_start(out=bc_t, in_=bass.AP(
                    tensor=bc_ext, offset=0, ap=[[0, 128], [1, NF]]))
            nc.gpsimd.iota(iota_i, pattern=[[1, NF]], base=0, channel_multiplier=0)
            nc.gpsimd.tensor_copy(iota_h, iota_i)
            # x block 0 lands first so tile 0 can start, then wc arrives
            # per chunk in consumption order (the bulk ring is in-order,
            # so tile 0 waits for ~1.3MB, not the whole 2MB of wc), then
            # the remaining x blocks. Batched DMAs — the sync queue is
            # issue-rate-bound at ~750ns per dma_start.
            xh_r = xh_ext.ap().rearrange("(c p) e -> p c e", p=128)
            xl_r = xl_ext.ap().rearrange("(c p) e -> p c e", p=128)
            wc_r = wc_ext.ap().rearrange("(c p) e -> p c e", p=128)
            xblocks = [256, 256] + [512] * ((BLOC - 512) // 512)
            start = 0
            for j, blk in enumerate(xblocks):
                cols = slice(start, start + blk)
                start += blk
                nc.sync.dma_start(out=xh_t[:, :, cols], in_=xh_r[:, :, cols])
                nc.sync.dma_start(out=xl_t[:, :, cols], in_=xl_r[:, :, cols])
                if j == 0:
                    for c in range(KC):
                        nc.sync.dma_start(out=wc_t[:, c, :],
                                          in_=wc_r[:, c, :])

            # ---- per-row-tile pipeline ----
            for i in range(NT):
                rows = slice(i * 128, (i + 1) * 128)

                # gate accumulates 3 bf16 hi/lo terms (xh@Wh + xh@Wl + xl@Wh;
                # the dropped xl@Wl term is O(2^-18)) into ps_g. The two xh
                # terms ride inside the main weight stream so their LDWEIGHTS
                # is the one already loaded for the 512-wide main matmuls;
                # only the 4 xl chunks pay a (reorder-window-hidden) reload.
                ps_g = psg.tile([128, 2 * E], F32)
                ps_h = [ps.tile([128, 1024], F32, tag="ps_half",
                                name=f"ps_half_{i}_{h}") for h in range(2)]
                # single interleaved pass: per chunk, 4x512-wide main
                # streams keep the PE ahead of the (duplicated, walrus
                # runs with ldw-opt off) LDWEIGHTS; the 3 tiny gate
                # matmuls ride along. (A split h0/h1 two-pass variant
                # measured 29% SLOWER: its half-length streams can't hide
                # 5 LDWs per chunk.)
                for c in range(KC):
                    for n in range(4):
                        nc.tensor.matmul(
                            ps_h[n // 2][:, (n % 2) * 512:(n % 2) * 512 + 512],
                            xh_t[:, c, rows],
                            wc_t[:, c, n * 512:(n + 1) * 512],
                            start=(c == 0), stop=(c == KC - 1))
                    nc.tensor.matmul(
                        ps_g[:, :], xh_t[:, c, rows],
                        whl_t[:, c, 0:2 * E],
                        start=(c == 0), stop=False)
                    nc.tensor.matmul(
                        ps_g[:, :], xh_t[:, c, rows],
                        whl_t[:, c, 2 * E:4 * E],
                        start=False, stop=False)
                    nc.tensor.matmul(
                        ps_g[:, :], xl_t[:, c, rows],
                        whl_t[:, c, 0:2 * E],
                        start=False, stop=(c == KC - 1))

                # h0 copy first on the scalar queue (frees psum buf early),
                # then the gate epilogue overlapping pass 2.
                y16_t = small.tile([128, NF], F16, tag="y16", bufs=3,
                                   name=f"y16_{i}")

                def copy_half(h):
                    src = ps_h[h]
                    if has_comp_bias:
                        src = small.tile([128, 1024], F32, tag="biased", bufs=3,
                                         name=f"biased_{i}_{h}")
                        nc.vector.tensor_tensor(
                            out=src, in0=ps_h[h],
                            in1=bc_t[:, h * 1024:(h + 1) * 1024], op=OP.add)
                    nc.scalar.activation(y16_t[:, h * 1024:(h + 1) * 1024],
                                         src, AF.Copy)

                copy_half(0)

                # gate epilogue -> threshold t = 128*(argmax+1)
                g_ps = ps_g[:, 0:E]
                n_ps = ps_g[:, E:2 * E]
                if has_gate_bias:
                    gn_t = small.tile([128, 2 * E], F32)
                    nc.vector.tensor_tensor(out=gn_t, in0=ps_g, in1=gb_t,
                                            op=OP.add)
                    g_ps, n_ps = gn_t[:, 0:E], gn_t[:, E:2 * E]
                # relu(z) as a single DVE tensor_scalar (max with 0) so the
                # scalar engine only carries Abs/Exp/Ln plus the big copies
                re_t = small.tile([128, E], F32)
                nc.vector.tensor_scalar(out=re_t, in0=n_ps, scalar1=0.0,
                                        scalar2=None, op0=OP.max)
                ab_t = small.tile([128, E], F32)
                nc.scalar.activation(ab_t, n_ps, AF.Abs)
                ex_t = small.tile([128, E], F32)
                nc.scalar.activation(ex_t, ab_t, AF.Exp, scale=-1.0)
                ln_t = small.tile([128, E], F32)
                nc.scalar.activation(ln_t, ex_t, AF.Ln, bias=1.0)
                # the two SBUF-only small ops go to GpSimd to unload DVE
                sp_t = small.tile([128, E], F32)
                nc.gpsimd.tensor_tensor(out=sp_t, in0=ln_t, in1=re_t, op=OP.add)
                he_t = small.tile([128, E], F32)
                nc.gpsimd.tensor_tensor(out=he_t, in0=sp_t, in1=eps_t, op=OP.mult)
                h_t = small.tile([128, E], F32)
                nc.vector.tensor_tensor(out=h_t, in0=he_t, in1=g_ps, op=OP.add)
                pm_t = small.tile([128, E], F32)
                nc.vector.tensor_tensor_scan(pm_t, h_t, h_t, initial=-1e30,
                                             op0=OP.max, op1=OP.bypass)
                bits_t = small.tile([128, E], F32)
                ks_t = small.tile([128, 1], F32)
                nc.vector.tensor_scalar(out=bits_t, in0=pm_t,
                                        scalar1=pm_t[:, E - 1:E], scalar2=0.0,
                                        op0=OP.is_lt, op1=OP.add, accum_out=ks_t)
                t_t = small.tile([128, 1], F32)
                nc.vector.tensor_scalar(out=t_t, in0=ks_t, scalar1=128.0,
                                        scalar2=128.0, op0=OP.mult, op1=OP.add)

                copy_half(1)

                # masked epilogue, restructured for DVE fast modes (the
                # fused scalar_tensor_tensor supports NO 2x/4x mode and
                # costs 1.5us per half from PSUM):
                #   mask16 = (iota16 < t)          DVE tensor_scalar, 4x_2p
                #   y16    = downcast(psum)        scalar engine, x2
                #   out    = mask16 * y16          DVE tensor_tensor, 2x_1p
                mask_t = small.tile([128, NF], F16, tag="mask", bufs=3,
                                    name=f"mask_{i}")
                nc.vector.tensor_scalar(out=mask_t, in0=iota_h,
                                        scalar1=t_t[:, 0:1], scalar2=None,
                                        op0=OP.is_lt)
                o_t = outp.tile([128, NF], F16)
                if i < NT - 1:
                    nc.vector.tensor_tensor(out=o_t, in0=mask_t, in1=y16_t,
                                            op=OP.mult)
                    nc.sync.dma_start(out=out_ext[rows, :], in_=o_t)
                else:
                    # last tile: split halves so h0's mult+DMA overlap the
                    # h1 psum copy, shortening the kernel tail
                    for h in range(2):
                        hc = slice(h * 1024, (h + 1) * 1024)
                        nc.vector.tensor_tensor(out=o_t[:, hc],
                                                in0=mask_t[:, hc],
                                                in1=y16_t[:, hc], op=OP.mult)
                        nc.sync.dma_start(out=out_ext[rows, hc],
                                          in_=o_t[:, hc])

    if split_waits:
        _split_multi_waits(nc)
    return nc


_NC_CACHE = {}


def kernel(x, Wc, bc, Wg_w, Wg_b, Wn_w, Wn_b, noise_eps):
    x = np.ascontiguousarray(np.asarray(x, dtype=np.float32))
    Wc = np.asarray(Wc, dtype=np.float32)
    bc = np.asarray(bc, dtype=np.float32)
    Wg_w = np.asarray(Wg_w, dtype=np.float32)
    Wg_b = np.asarray(Wg_b, dtype=np.float32)
    Wn_w = np.asarray(Wn_w, dtype=np.float32)
    Wn_b = np.asarray(Wn_b, dtype=np.float32)
    noise_eps = np.asarray(noise_eps, dtype=np.float32)

    has_gate_bias = bool(np.any(Wg_b) or np.any(Wn_b))
    has_comp_bias = bool(np.any(bc))

    key = (has_gate_bias, has_comp_bias)
    if key not in _NC_CACHE:
        _NC_CACHE[key] = _build(has_gate_bias, has_comp_bias)
    nc = _NC_CACHE[key]

    bf = ml_dtypes.bfloat16
    xT = x.T                                   # [D, B]
    xh = xT.astype(bf)
    xl = (xT - xh.astype(np.float32)).astype(bf)
    wgn = np.concatenate([Wg_w, Wn_w], axis=1)  # [D, 2E] fp32
    wgh = wgn.astype(bf)
    wgl = (wgn - wgh.astype(np.float32)).astype(bf)
    whl = np.ascontiguousarray(np.concatenate([wgh, wgl], axis=1))  # [D, 4E]
    wc_flat = np.ascontiguousarray(
        Wc.transpose(1, 0, 2).reshape(D, NF).astype(bf))
    eps2 = np.ascontiguousarray(noise_eps.reshape(1, E))

    in_maps = []
    for i in range(N_CORES):
        cols = slice(i * BLOC, (i + 1) * BLOC)
        m = {
            "xh": np.ascontiguousarray(xh[:, cols]),
            "xl": np.ascontiguousarray(xl[:, cols]),
            "wc": wc_flat,
            "whl": whl,
            "eps": eps2,
        }
        if has_gate_bias:
            m["gb"] = np.ascontiguousarray(
                np.concatenate([Wg_b, Wn_b]).reshape(1, 2 * E).astype(np.float32))
        if has_comp_bias:
            m["bc"] = np.ascontiguousarray(bc.reshape(1, NF).astype(np.float32))
        in_maps.append(m)

    res = run_bass_kernel_spmd(nc, in_maps, core_ids=list(range(N_CORES)))
    out = np.concatenate(
        [np.asarray(res.results[i]["out"]).astype(np.float32)
         for i in range(N_CORES)], axis=0)
    return out

